# revision 5
# baseline (speedup 1.0000x reference)
"""Bahdanau-style sparse attention kernel for Trainium2, data-parallel over
batch on 8 cores.

Math (per batch row g):
    h_proj = hidden @ Wh.T + b_attn                      [128]
    energy[s, :] = tanh(h_proj + embs[s] @ We.T)         [S, 128]
    att[s] = v . energy[s, :]                            [S]
    out = softmax_S(where(mask==0, -1e10, att))

Sparsity: masked positions (mask==0, ~50%) contribute exactly 0, so the
host compacts each row to its unmasked columns. Rows are sorted by
count and grouped into 8 slots of 8 (one row per core per slot); slot
width Ws[b] = 128-ceil of the group max, so the single SPMD NEFF fits
all cores with ~3% padding instead of a global max.

Per-core device schedule (8 slots, Ws[b] compacted columns each):
  - ACT (tanh) is the roofline: ~16.9K cols @ 0.83ns/col. Everything
    else is arranged to keep ACT gap-free from its first tile on.
  - DMA: first transfer carries the weights AND the first 512 cols of
    slot 0 (one issue, ~0.3us payload), then ramped col-range chunks of
    the flat embsT [128, Wtot]; issues split across SP and DVE
    sequencers so issue latency (~0.6us each) never gates the stream.
  - PE: warm-up matmuls on memset zeros start immediately (HAM p-state
    ramp), then h_projT = WhT.T@hiddenT + b_attn (bias via a K=1
    matmul), then per-tile We-matmuls in 512-col pieces; one-hot-column
    v-matmuls scatter each (slot, 512-chunk) att row into a persistent
    [NP, 512] PSUM accumulator pre-seeded with the pad bias (-30) via
    an identity matmul. Slot 0's first tile runs tanh in 512-col pieces
    for the earliest possible ACT start.
  - Pool: memsets + the PSUM->SBUF h_proj move (off the ACT/DVE paths).
  - Epilogue: exp with accum_out partials, block-ones matmul for the
    per-row denominators, DVE reciprocal + scale, single out DMA.
  - Softmax skips max-subtraction: |att| <= ||v||_1 ~ 5.7 so exp is
    safe; pad bias of -30 keeps padded columns out of the denominator.
"""

import numpy as np

B = 64
S = 4096
D = 128  # dec_dim == emb_dim == 128
NCORES = 8
BPC = B // NCORES  # 8 batch rows (slots) per core
FW = 512  # att accumulator chunk width (= one PSUM bank of f32)
TW = 1536  # e_proj / tanh tile width (3 PSUM banks)
GRAN = 128  # per-slot width granularity

_COMPILED = {}


def _row_tiles(w):
    """Split a slot width into TW-aligned psum tiles."""
    out = []
    off = 0
    while off < w:
        t = min(TW, w - off)
        out.append((off, t))
        off += t
    return out


def _build_bass(Ws):
    import concourse.bacc as bacc
    import concourse.mybir as mybir
    from concourse.tile import TileContext

    f32 = mybir.dt.float32
    fp16 = mybir.dt.float16
    AF = mybir.ActivationFunctionType

    Ws = list(Ws)
    Wtot = sum(Ws)
    offs = [0]
    for w in Ws:
        offs.append(offs[-1] + w)
    cpbs = [-(-w // FW) for w in Ws]  # chunks per slot
    pbase = [0]
    for c in cpbs:
        pbase.append(pbase[-1] + c)
    NP = pbase[-1]

    R0W = min(FW, Ws[0])  # slot-0 head columns carried in fpmain

    # fpmain cols: WeT | WhT | hiddenT | b_attn row (p0) | ones row (p0) | r0head
    FPW = D + D + BPC + D + BPC + R0W
    AUXW = FW + NP + 2 * NP  # maskC | idNP | vstrip

    nc = bacc.Bacc(
        "TRN2", target_bir_lowering=False, debug=False, num_devices=NCORES
    )

    embsT = nc.dram_tensor("embsT", [D, Wtot], fp16, kind="ExternalInput")
    fpmain = nc.dram_tensor("fpmain", [D, FPW], fp16, kind="ExternalInput")
    fpaux = nc.dram_tensor("fpaux", [D, AUXW], fp16, kind="ExternalInput")
    bones = nc.dram_tensor("bones", [NP, NP], f32, kind="ExternalInput")
    out_d = nc.dram_tensor("out", [NP, FW], f32, kind="ExternalOutput")

    # embs DMA chunk plan over cols [R0W, Wtot): ramped sizes, first ones
    # small so compute starts early, later ones big to amortize issue cost.
    ramp = [512, 1024, 1536, 2048, 2560, 3072, 3584]
    chunks = []
    off = R0W
    ri = 0
    while off < Wtot:
        w = ramp[ri] if ri < len(ramp) else 4096
        ri += 1
        w = min(w, Wtot - off)
        chunks.append((off, w))
        off += w
    sp_chunks = chunks[:4]
    dve_chunks = chunks[4:]

    with TileContext(nc) as tc:
        with (
            tc.tile_pool(name="consts", bufs=1) as consts,
            tc.tile_pool(name="embs", bufs=1) as embs_pool,
            tc.tile_pool(name="energy", bufs=4) as energy_pool,
            tc.tile_pool(name="post", bufs=1) as post,
            tc.tile_pool(name="ps_big", bufs=2, space="PSUM") as ps_big,
            tc.tile_pool(name="ps_att", bufs=1, space="PSUM") as ps_att,
        ):
            # --- input DMAs ---
            fpmain_sb = consts.tile([D, FPW], fp16)
            nc.sync.dma_start(out=fpmain_sb, in_=fpmain[:, :])
            o = 0
            WeT_sb = fpmain_sb[:, o : o + D]; o += D
            WhT_sb = fpmain_sb[:, o : o + D]; o += D
            hT_sb = fpmain_sb[:, o : o + BPC]; o += BPC
            brow_sb = fpmain_sb[0:1, o : o + D]; o += D
            ones_sb = fpmain_sb[0:1, o : o + BPC]; o += BPC
            r0head_sb = fpmain_sb[:, o : o + R0W]

            embsT_sb = embs_pool.tile([D, Wtot], fp16)
            for off, w in sp_chunks:
                nc.sync.dma_start(out=embsT_sb[:, off : off + w],
                                  in_=embsT[:, off : off + w])

            # DVE: warm-up memsets (DVE is otherwise idle until the epilogue)
            wu_w = consts.tile([D, D], fp16)
            nc.vector.memset(wu_w[:, :], 0.0)
            wu_m = consts.tile([D, FW], fp16)
            nc.vector.memset(wu_m[:, :], 0.0)

            # Pool: ACT-table dummy first, then its share of the DMA issues
            # (SWDGE — Pool is idle otherwise; keeps the SP issue chain short)
            dummy = consts.tile([1, 8], f32)
            nc.gpsimd.memset(dummy[:, :], 0.0)
            fpaux_sb = consts.tile([D, AUXW], fp16)
            nc.gpsimd.dma_start(out=fpaux_sb, in_=fpaux[:, :])

            maskC_sb = fpaux_sb[0:NP, 0:FW]
            idNP_sb = fpaux_sb[0:NP, FW : FW + NP]
            vstrip_sb = fpaux_sb[:, FW + NP : FW + 3 * NP]

            # ACT table preload on a dummy (off the critical path)
            dummy2 = consts.tile([1, 8], f32)
            nc.scalar.activation(out=dummy2[:, :], in_=dummy[:, :], func=AF.Tanh)

            # PE warm-up on zero tiles: keeps the HAM p-state ramping while
            # the DMA head streams in.
            for _ in range(4):
                wu_ps = ps_big.tile([D, TW], f32, tag="ps")
                nc.tensor.matmul(wu_ps[:, 0:FW], wu_w[:, :], wu_m[:, :])

            # h_projT[d, b] = sum_k WhT[k, d]*hiddenT[k, b] + b_attn[d]
            # (bias added via a K=1 matmul on a ones row)
            hp_ps = ps_big.tile([D, TW], f32, tag="ps")
            nc.tensor.matmul(hp_ps[:, 0:BPC], WhT_sb[:, :], hT_sb[:, :],
                             start=True, stop=False, skip_group_check=True)
            nc.tensor.matmul(hp_ps[:, 0:BPC], brow_sb[:, :], ones_sb[:, :],
                             start=False, stop=True, skip_group_check=True)
            # PSUM -> SBUF move for the tanh bias (gpsimd can't read PSUM;
            # one tiny ACT copy before the tanh stream starts)
            hprojT_sb = consts.tile([D, BPC], f32)
            nc.scalar.copy(hprojT_sb[:, :], hp_ps[:, 0:BPC])
            # Pool: remaining embs chunks + epilogue consts via SWDGE
            for off, w in dve_chunks:
                nc.gpsimd.dma_start(out=embsT_sb[:, off : off + w],
                                    in_=embsT[:, off : off + w])
            bones_sb = consts.tile([NP, NP], f32)
            nc.gpsimd.dma_start(out=bones_sb, in_=bones[:, :])

            # att accumulator [NP, FW]: partition pbase[b]+j holds slot b's
            # cols [FW*j, FW*(j+1)); seeded with the pad/mask bias via
            # I @ maskC.
            att_ps = ps_att.tile([NP, FW], f32)
            n_mm_tot = 1 + NP
            n_vmm = 0
            seeded = False

            def emit_seed():
                nonlocal n_vmm, seeded
                nc.tensor.matmul(
                    att_ps[:, :], idNP_sb[:, :], maskC_sb[:, :],
                    start=True, stop=False, skip_group_check=True,
                )
                n_vmm += 1
                seeded = True

            def emit_vmms(pending):
                nonlocal n_vmm
                if pending and not seeded:
                    emit_seed()
                for en_t, b, toff, tw in pending:
                    co = 0
                    while co < tw:
                        cw = min(FW, tw - co)
                        p = pbase[b] + (toff + co) // FW
                        nc.tensor.matmul(
                            att_ps[:, 0:cw],
                            vstrip_sb[:, NP - p : 2 * NP - p],
                            en_t[:, co : co + cw],
                            start=False, stop=(n_vmm >= n_mm_tot - 1),
                            skip_group_check=True,
                        )
                        n_vmm += 1
                        co += cw

            def do_tile(b, toff, tw, fine):
                pe_t = ps_big.tile([D, TW], f32, tag="ps")
                mo = 0
                while mo < tw:
                    mw = min(FW, tw - mo)
                    src_off = toff + mo
                    if b == 0 and src_off < R0W:
                        src = r0head_sb[:, src_off : src_off + mw]
                    else:
                        src = embsT_sb[:, offs[b] + src_off : offs[b] + src_off + mw]
                    nc.tensor.matmul(pe_t[:, mo : mo + mw], WeT_sb[:, :], src)
                    mo += mw
                en_t = energy_pool.tile([D, TW], fp16, tag="en")
                ao = 0
                aw_step = FW if fine else tw
                while ao < tw:
                    aw = min(aw_step, tw - ao)
                    nc.scalar.activation(
                        out=en_t[:, ao : ao + aw],
                        in_=pe_t[:, ao : ao + aw],
                        func=AF.Tanh,
                        bias=hprojT_sb[:, b : b + 1],
                        scale=1.0,
                    )
                    ao += aw
                return (en_t, b, toff, tw)

            prev = []
            for b in range(BPC):
                cur = []
                for ti, (toff, tw) in enumerate(_row_tiles(Ws[b])):
                    cur.append(do_tile(b, toff, tw, fine=(b == 0 and ti == 0)))
                    if ti == 0:
                        emit_vmms(prev)
                        prev = []
                prev.extend(cur)
            emit_vmms(prev)

            # softmax: p = exp(att) with per-partition (chunk) partial sums
            p_sb = post.tile([NP, FW], f32)
            partials_sb = post.tile([NP, 1], f32)
            nc.scalar.activation(
                out=p_sb[:, :], in_=att_ps[:, :], func=AF.Exp,
                accum_out=partials_sb[:, 0:1],
            )
            # block-ones matmul: den[p] = sum of partials over p's slot
            den_ps = ps_big.tile([D, TW], f32, tag="ps")
            nc.tensor.matmul(den_ps[0:NP, 0:1], bones_sb[:, :], partials_sb[:, 0:1])
            recip_sb = post.tile([NP, 1], f32)
            nc.vector.reciprocal(recip_sb[:, :], den_ps[0:NP, 0:1])

            out_sb = post.tile([NP, FW], f32)
            nc.vector.tensor_scalar_mul(out_sb[:, :], p_sb[:, :], recip_sb[:, 0:1])
            nc.sync.dma_start(out=out_d[:, :], in_=out_sb[:, :])

    nc.compile()
    return nc


def _get_nc(Ws):
    if Ws not in _COMPILED:
        _COMPILED[Ws] = _build_bass(Ws)
    return _COMPILED[Ws]


def _plan(mask):
    """Slot-balanced row assignment shared by all cores (single NEFF)."""
    cnts = [int(np.count_nonzero(mask[g])) for g in range(B)]
    order = sorted(range(B), key=lambda g: -cnts[g])
    assign = [[0] * BPC for _ in range(NCORES)]  # [core][slot] -> row
    Ws = []
    for b in range(BPC):
        grp = order[b * NCORES : (b + 1) * NCORES]
        for c in range(NCORES):
            assign[c][b] = grp[c]
        mx = max(cnts[g] for g in grp)
        Ws.append(max(GRAN, -(-mx // GRAN) * GRAN))
    return cnts, assign, tuple(Ws)


def _prep(hidden, seq_embs, mask, W_attn, b_attn, v_w):
    """Host-side prep: slot assignment, mask compaction (gather), fp16
    cast, relayouts. All FLOPs on the model data happen on device."""
    hidden = np.asarray(hidden, dtype=np.float32)
    seq_embs = np.asarray(seq_embs, dtype=np.float32)
    mask = np.asarray(mask)
    W_attn = np.asarray(W_attn, dtype=np.float32)
    b_attn = np.asarray(b_attn, dtype=np.float32)
    v_w = np.asarray(v_w, dtype=np.float32)

    cnts, assign, Ws = _plan(mask)
    Wtot = sum(Ws)
    offs = np.concatenate([[0], np.cumsum(Ws)]).astype(int)
    cpbs = [-(-w // FW) for w in Ws]
    pbase = np.concatenate([[0], np.cumsum(cpbs)]).astype(int)
    NP = int(pbase[-1])
    R0W = min(FW, Ws[0])
    FPW = D + D + BPC + D + BPC + R0W
    AUXW = FW + NP + 2 * NP

    WeT = np.ascontiguousarray(W_attn[:, D:].T.astype(np.float16))
    WhT = np.ascontiguousarray(W_attn[:, :D].T.astype(np.float16))
    vstrip = np.zeros((D, 2 * NP), dtype=np.float16)
    vstrip[:, NP] = v_w[0].astype(np.float16)
    bones = np.zeros((NP, NP), dtype=np.float32)
    for b in range(BPC):
        bones[pbase[b] : pbase[b + 1], pbase[b] : pbase[b + 1]] = 1.0

    seq16 = seq_embs.astype(np.float16)  # [S, B, D]
    idxs_all = [np.flatnonzero(mask[g]) for g in range(B)]

    in_maps = []
    for c in range(NCORES):
        embsT = np.zeros((D, Wtot), dtype=np.float16)
        maskC = np.full((NP, FW), -30.0, dtype=np.float16)
        for b in range(BPC):
            g = assign[c][b]
            ix = idxs_all[g]
            cnt = len(ix)
            if cnt:
                embsT[:, offs[b] : offs[b] + cnt] = seq16[ix, g, :].T
            for j in range(cpbs[b]):
                lo = j * FW
                real = min(max(cnt - lo, 0), FW)
                if real:
                    maskC[pbase[b] + j, :real] = 0.0
        hiddenT = np.ascontiguousarray(
            hidden[[assign[c][b] for b in range(BPC)]].T
        ).astype(np.float16)
        fpmain = np.zeros((D, FPW), dtype=np.float16)
        o = 0
        fpmain[:, o : o + D] = WeT; o += D
        fpmain[:, o : o + D] = WhT; o += D
        fpmain[:, o : o + BPC] = hiddenT; o += BPC
        fpmain[0, o : o + D] = b_attn.astype(np.float16); o += D
        fpmain[0, o : o + BPC] = 1.0; o += BPC
        fpmain[:, o : o + R0W] = embsT[:, :R0W]
        fpaux = np.zeros((D, AUXW), dtype=np.float16)
        fpaux[:NP, 0:FW] = maskC
        fpaux[:NP, FW : FW + NP] = np.eye(NP, dtype=np.float16)
        fpaux[:, FW + NP : FW + 3 * NP] = vstrip
        in_maps.append(
            {"embsT": embsT, "fpmain": fpmain, "fpaux": fpaux, "bones": bones}
        )
    return Ws, cnts, assign, pbase, cpbs, in_maps


def kernel(hidden, seq_embs, mask, W_attn, b_attn, v_w, **run_kwargs):
    from concourse.bass_utils import run_bass_kernel_spmd

    Ws, cnts, assign, pbase, cpbs, in_maps = _prep(
        hidden, seq_embs, mask, W_attn, b_attn, v_w
    )
    nc = _get_nc(Ws)
    res = run_bass_kernel_spmd(
        nc, in_maps, core_ids=list(range(NCORES)), **run_kwargs
    )
    mask = np.asarray(mask)
    idxs_all = [np.flatnonzero(mask[g]) for g in range(B)]
    out = np.zeros((B, S), dtype=np.float32)
    for c in range(NCORES):
        comp = res.results[c]["out"].astype(np.float32)  # [NP, FW]
        for b in range(BPC):
            g = assign[c][b]
            ix = idxs_all[g]
            cnt = len(ix)
            if cnt:
                flat = comp[pbase[b] : pbase[b] + cpbs[b], :].reshape(-1)
                out[g, ix] = flat[:cnt]
            else:
                out[g, :] = 1.0 / S  # softmax of all -1e10 is uniform
    if run_kwargs:
        kernel.last_results = res  # stash for the profiling harness
    return out


# revision 9
# speedup vs baseline: 1.1250x; 1.1250x over previous
"""Bahdanau-style sparse attention kernel for Trainium2, data-parallel over
batch on 8 cores.

Math (per batch row g):
    h_proj = hidden @ Wh.T + b_attn                      [128]
    energy[s, :] = tanh(h_proj + embs[s] @ We.T)         [S, 128]
    att[s] = v . energy[s, :]                            [S]
    out = softmax_S(where(mask==0, -1e10, att))

Sparsity: masked positions (mask==0, ~50%) contribute exactly 0, so the
host compacts each row to its unmasked columns. Rows are sorted by
count and grouped into 8 slots of 8 (one row per core per slot); slot
width Ws[b] = 128-ceil of the group max, so the single SPMD NEFF fits
all cores with ~3% padding instead of a global max.

Per-core device schedule (8 slots, Ws[b] compacted columns each):
  - ACT (tanh) is the roofline: ~16.9K cols @ 0.83ns/col. Everything
    else is arranged to keep ACT gap-free from its first tile on.
  - DMA: first transfer carries the weights AND the first 512 cols of
    slot 0 (one issue, ~0.3us payload), then ramped col-range chunks of
    the flat embsT [128, Wtot]; issues split across SP and DVE
    sequencers so issue latency (~0.6us each) never gates the stream.
  - PE: warm-up matmuls on memset zeros start immediately (HAM p-state
    ramp), then h_projT = WhT.T@hiddenT + b_attn (bias via a K=1
    matmul), then per-tile We-matmuls in 512-col pieces; one-hot-column
    v-matmuls scatter each (slot, 512-chunk) att row into a persistent
    [NP, 512] PSUM accumulator pre-seeded with the pad bias (-30) via
    an identity matmul. Slot 0's first tile runs tanh in 512-col pieces
    for the earliest possible ACT start.
  - Pool: memsets + the PSUM->SBUF h_proj move (off the ACT/DVE paths).
  - Epilogue: exp with accum_out partials, block-ones matmul for the
    per-row denominators, DVE reciprocal + scale, single out DMA.
  - Softmax skips max-subtraction: |att| <= ||v||_1 ~ 5.7 so exp is
    safe; pad bias of -30 keeps padded columns out of the denominator.
"""

import numpy as np

B = 64
S = 4096
D = 128  # dec_dim == emb_dim == 128
NCORES = 8
BPC = B // NCORES  # 8 batch rows (slots) per core
FW = 512  # att accumulator chunk width (= one PSUM bank of f32)
TW = 1536  # e_proj / tanh tile width (3 PSUM banks)
GRAN = 128  # per-slot width granularity

_COMPILED = {}


def _row_tiles(w):
    """Split a slot width into TW-aligned psum tiles."""
    out = []
    off = 0
    while off < w:
        t = min(TW, w - off)
        out.append((off, t))
        off += t
    return out


def _build_bass(Ws):
    import concourse.bacc as bacc
    import concourse.mybir as mybir
    from concourse.tile import TileContext

    f32 = mybir.dt.float32
    fp16 = mybir.dt.float16
    AF = mybir.ActivationFunctionType

    Ws = list(Ws)
    Wtot = sum(Ws)
    offs = [0]
    for w in Ws:
        offs.append(offs[-1] + w)
    cpbs = [-(-w // FW) for w in Ws]  # chunks per slot
    pbase = [0]
    for c in cpbs:
        pbase.append(pbase[-1] + c)
    NP = pbase[-1]

    R0W = min(FW, Ws[0])  # slot-0 head columns carried in fpmain

    # fpmain cols: WeT | WhT | hiddenT | b_attn row (p0) | ones row (p0) | r0head
    FPW = D + D + BPC + D + BPC + R0W
    AUXW = FW + NP + 2 * NP  # maskC | idNP | vstrip

    nc = bacc.Bacc(
        "TRN2", target_bir_lowering=False, debug=False, num_devices=NCORES
    )

    embsT = nc.dram_tensor("embsT", [D, Wtot], fp16, kind="ExternalInput")
    fpmain = nc.dram_tensor("fpmain", [D, FPW], fp16, kind="ExternalInput")
    fpaux = nc.dram_tensor("fpaux", [D, AUXW], fp16, kind="ExternalInput")
    bones = nc.dram_tensor("bones", [NP, NP], f32, kind="ExternalInput")
    out_d = nc.dram_tensor("out", [NP, FW], f32, kind="ExternalOutput")

    # embs DMA chunk plan over cols [R0W, Wtot): ramped sizes, first ones
    # small so compute starts early, later ones big to amortize issue cost.
    ramp = [512, 1024, 1536, 2048, 2560, 3072, 3584]
    chunks = []
    off = R0W
    ri = 0
    while off < Wtot:
        w = ramp[ri] if ri < len(ramp) else 4096
        ri += 1
        w = min(w, Wtot - off)
        chunks.append((off, w))
        off += w


    with TileContext(nc) as tc:
        with (
            tc.tile_pool(name="consts", bufs=1) as consts,
            tc.tile_pool(name="embs", bufs=1) as embs_pool,
            tc.tile_pool(name="energy", bufs=4) as energy_pool,
            tc.tile_pool(name="post", bufs=1) as post,
            tc.tile_pool(name="ps_big", bufs=2, space="PSUM") as ps_big,
            tc.tile_pool(name="ps_att", bufs=1, space="PSUM") as ps_att,
        ):
            # --- input DMAs ---
            fpmain_sb = consts.tile([D, FPW], fp16)
            nc.sync.dma_start(out=fpmain_sb, in_=fpmain[:, :])
            o = 0
            WeT_sb = fpmain_sb[:, o : o + D]; o += D
            WhT_sb = fpmain_sb[:, o : o + D]; o += D
            hT_sb = fpmain_sb[:, o : o + BPC]; o += BPC
            brow_sb = fpmain_sb[0:1, o : o + D]; o += D
            ones_sb = fpmain_sb[0:1, o : o + BPC]; o += BPC
            r0head_sb = fpmain_sb[:, o : o + R0W]

            # fpaux rides the ACT sequencer (idle before the tanh stream;
            # one 667ns issue, its HWDGE ring runs concurrently with SP's)
            fpaux_sb = consts.tile([D, AUXW], fp16)
            nc.scalar.dma_start(out=fpaux_sb, in_=fpaux[:, :])

            embsT_sb = embs_pool.tile([D, Wtot], fp16)
            for off, w in chunks:
                nc.sync.dma_start(out=embsT_sb[:, off : off + w],
                                  in_=embsT[:, off : off + w])
            bones_sb = consts.tile([NP, NP], f32)
            nc.sync.dma_start(out=bones_sb, in_=bones[:, :])

            # DVE: warm-up memsets (DVE is otherwise idle until the epilogue)
            wu_w = consts.tile([D, D], fp16)
            nc.vector.memset(wu_w[:, :], 0.0)
            wu_m = consts.tile([D, FW], fp16)
            nc.vector.memset(wu_m[:, :], 0.0)

            dummy = consts.tile([1, 8], f32)
            nc.gpsimd.memset(dummy[:, :], 0.0)

            maskC_sb = fpaux_sb[0:NP, 0:FW]
            idNP_sb = fpaux_sb[0:NP, FW : FW + NP]
            vstrip_sb = fpaux_sb[:, FW + NP : FW + 3 * NP]

            # ACT table preload on a dummy (off the critical path)
            dummy2 = consts.tile([1, 8], f32)
            nc.scalar.activation(out=dummy2[:, :], in_=dummy[:, :], func=AF.Tanh)

            # PE warm-up on zero tiles: keeps the HAM p-state ramping while
            # the DMA head streams in.
            for _ in range(4):
                wu_ps = ps_big.tile([D, TW], f32, tag="ps")
                nc.tensor.matmul(wu_ps[:, 0:FW], wu_w[:, :], wu_m[:, :])

            # h_projT[d, b] = sum_k WhT[k, d]*hiddenT[k, b] + b_attn[d]
            # (bias added via a K=1 matmul on a ones row)
            hp_ps = ps_big.tile([D, TW], f32, tag="ps")
            nc.tensor.matmul(hp_ps[:, 0:BPC], WhT_sb[:, :], hT_sb[:, :],
                             start=True, stop=False, skip_group_check=True)
            nc.tensor.matmul(hp_ps[:, 0:BPC], brow_sb[:, :], ones_sb[:, :],
                             start=False, stop=True, skip_group_check=True)
            # PSUM -> SBUF move for the tanh bias (gpsimd can't read PSUM;
            # one tiny ACT copy before the tanh stream starts)
            hprojT_sb = consts.tile([D, BPC], f32)
            nc.scalar.copy(hprojT_sb[:, :], hp_ps[:, 0:BPC])

            # att accumulator [NP, FW]: partition pbase[b]+j holds slot b's
            # cols [FW*j, FW*(j+1)); seeded with the pad/mask bias via
            # I @ maskC.
            att_ps = ps_att.tile([NP, FW], f32)
            n_mm_tot = 1 + NP
            n_vmm = 0
            seeded = False

            def emit_seed():
                nonlocal n_vmm, seeded
                nc.tensor.matmul(
                    att_ps[:, :], idNP_sb[:, :], maskC_sb[:, :],
                    start=True, stop=False, skip_group_check=True,
                )
                n_vmm += 1
                seeded = True

            def emit_vmms(pending):
                nonlocal n_vmm
                if pending and not seeded:
                    emit_seed()
                for en_t, b, toff, tw in pending:
                    co = 0
                    while co < tw:
                        cw = min(FW, tw - co)
                        p = pbase[b] + (toff + co) // FW
                        nc.tensor.matmul(
                            att_ps[:, 0:cw],
                            vstrip_sb[:, NP - p : 2 * NP - p],
                            en_t[:, co : co + cw],
                            start=False, stop=(n_vmm >= n_mm_tot - 1),
                            skip_group_check=True,
                        )
                        n_vmm += 1
                        co += cw

            def do_tile(b, toff, tw, fine):
                pe_t = ps_big.tile([D, TW], f32, tag="ps")
                mo = 0
                while mo < tw:
                    mw = min(FW, tw - mo)
                    src_off = toff + mo
                    if b == 0 and src_off < R0W:
                        src = r0head_sb[:, src_off : src_off + mw]
                    else:
                        src = embsT_sb[:, offs[b] + src_off : offs[b] + src_off + mw]
                    nc.tensor.matmul(pe_t[:, mo : mo + mw], WeT_sb[:, :], src)
                    mo += mw
                en_t = energy_pool.tile([D, TW], fp16, tag="en")
                ao = 0
                aw_step = FW if fine else tw
                while ao < tw:
                    aw = min(aw_step, tw - ao)
                    nc.scalar.activation(
                        out=en_t[:, ao : ao + aw],
                        in_=pe_t[:, ao : ao + aw],
                        func=AF.Tanh,
                        bias=hprojT_sb[:, b : b + 1],
                        scale=1.0,
                    )
                    ao += aw
                return (en_t, b, toff, tw)

            # Interleave with a two-tile lag: PE order is
            #   eproj(k), vmms(k-2), eproj(k+1), vmms(k-1), ...
            # so a vmm batch only ever waits on a tanh that finished while
            # the previous eproj ran — PE never stalls ACT.
            all_tiles = [
                (b, toff, tw, (b == 0 and ti == 0))
                for b in range(BPC)
                for ti, (toff, tw) in enumerate(_row_tiles(Ws[b]))
            ]
            window = []
            for b, toff, tw, fine in all_tiles:
                window.append(do_tile(b, toff, tw, fine))
                if len(window) > 2:
                    emit_vmms([window.pop(0)])
            emit_vmms(window)

            # softmax: p = exp(att) with per-partition (chunk) partial sums
            p_sb = post.tile([NP, FW], f32)
            partials_sb = post.tile([NP, 1], f32)
            nc.scalar.activation(
                out=p_sb[:, :], in_=att_ps[:, :], func=AF.Exp,
                accum_out=partials_sb[:, 0:1],
            )
            # block-ones matmul: den[p] = sum of partials over p's slot
            den_ps = ps_big.tile([D, TW], f32, tag="ps")
            nc.tensor.matmul(den_ps[0:NP, 0:1], bones_sb[:, :], partials_sb[:, 0:1])
            recip_sb = post.tile([NP, 1], f32)
            nc.vector.reciprocal(recip_sb[:, :], den_ps[0:NP, 0:1])

            out_sb = post.tile([NP, FW], f32)
            nc.vector.tensor_scalar_mul(out_sb[:, :], p_sb[:, :], recip_sb[:, 0:1])
            nc.sync.dma_start(out=out_d[:, :], in_=out_sb[:, :])

    nc.compile()
    return nc


def _get_nc(Ws):
    if Ws not in _COMPILED:
        _COMPILED[Ws] = _build_bass(Ws)
    return _COMPILED[Ws]


def _plan(mask):
    """Slot-balanced row assignment shared by all cores (single NEFF)."""
    cnts = [int(np.count_nonzero(mask[g])) for g in range(B)]
    order = sorted(range(B), key=lambda g: -cnts[g])
    assign = [[0] * BPC for _ in range(NCORES)]  # [core][slot] -> row
    Ws = []
    for b in range(BPC):
        grp = order[b * NCORES : (b + 1) * NCORES]
        for c in range(NCORES):
            assign[c][b] = grp[c]
        mx = max(cnts[g] for g in grp)
        Ws.append(max(GRAN, -(-mx // GRAN) * GRAN))
    return cnts, assign, tuple(Ws)


def _prep(hidden, seq_embs, mask, W_attn, b_attn, v_w):
    """Host-side prep: slot assignment, mask compaction (gather), fp16
    cast, relayouts. All FLOPs on the model data happen on device."""
    hidden = np.asarray(hidden, dtype=np.float32)
    seq_embs = np.asarray(seq_embs, dtype=np.float32)
    mask = np.asarray(mask)
    W_attn = np.asarray(W_attn, dtype=np.float32)
    b_attn = np.asarray(b_attn, dtype=np.float32)
    v_w = np.asarray(v_w, dtype=np.float32)

    cnts, assign, Ws = _plan(mask)
    Wtot = sum(Ws)
    offs = np.concatenate([[0], np.cumsum(Ws)]).astype(int)
    cpbs = [-(-w // FW) for w in Ws]
    pbase = np.concatenate([[0], np.cumsum(cpbs)]).astype(int)
    NP = int(pbase[-1])
    R0W = min(FW, Ws[0])
    FPW = D + D + BPC + D + BPC + R0W
    AUXW = FW + NP + 2 * NP

    WeT = np.ascontiguousarray(W_attn[:, D:].T.astype(np.float16))
    WhT = np.ascontiguousarray(W_attn[:, :D].T.astype(np.float16))
    vstrip = np.zeros((D, 2 * NP), dtype=np.float16)
    vstrip[:, NP] = v_w[0].astype(np.float16)
    bones = np.zeros((NP, NP), dtype=np.float32)
    for b in range(BPC):
        bones[pbase[b] : pbase[b + 1], pbase[b] : pbase[b + 1]] = 1.0

    seq16 = seq_embs.astype(np.float16)  # [S, B, D]
    idxs_all = [np.flatnonzero(mask[g]) for g in range(B)]

    in_maps = []
    for c in range(NCORES):
        embsT = np.zeros((D, Wtot), dtype=np.float16)
        maskC = np.full((NP, FW), -30.0, dtype=np.float16)
        for b in range(BPC):
            g = assign[c][b]
            ix = idxs_all[g]
            cnt = len(ix)
            if cnt:
                embsT[:, offs[b] : offs[b] + cnt] = seq16[ix, g, :].T
            for j in range(cpbs[b]):
                lo = j * FW
                real = min(max(cnt - lo, 0), FW)
                if real:
                    maskC[pbase[b] + j, :real] = 0.0
        hiddenT = np.ascontiguousarray(
            hidden[[assign[c][b] for b in range(BPC)]].T
        ).astype(np.float16)
        fpmain = np.zeros((D, FPW), dtype=np.float16)
        o = 0
        fpmain[:, o : o + D] = WeT; o += D
        fpmain[:, o : o + D] = WhT; o += D
        fpmain[:, o : o + BPC] = hiddenT; o += BPC
        fpmain[0, o : o + D] = b_attn.astype(np.float16); o += D
        fpmain[0, o : o + BPC] = 1.0; o += BPC
        fpmain[:, o : o + R0W] = embsT[:, :R0W]
        fpaux = np.zeros((D, AUXW), dtype=np.float16)
        fpaux[:NP, 0:FW] = maskC
        fpaux[:NP, FW : FW + NP] = np.eye(NP, dtype=np.float16)
        fpaux[:, FW + NP : FW + 3 * NP] = vstrip
        in_maps.append(
            {"embsT": embsT, "fpmain": fpmain, "fpaux": fpaux, "bones": bones}
        )
    return Ws, cnts, assign, pbase, cpbs, in_maps


def kernel(hidden, seq_embs, mask, W_attn, b_attn, v_w, **run_kwargs):
    from concourse.bass_utils import run_bass_kernel_spmd

    Ws, cnts, assign, pbase, cpbs, in_maps = _prep(
        hidden, seq_embs, mask, W_attn, b_attn, v_w
    )
    nc = _get_nc(Ws)
    res = run_bass_kernel_spmd(
        nc, in_maps, core_ids=list(range(NCORES)), **run_kwargs
    )
    mask = np.asarray(mask)
    idxs_all = [np.flatnonzero(mask[g]) for g in range(B)]
    out = np.zeros((B, S), dtype=np.float32)
    for c in range(NCORES):
        comp = res.results[c]["out"].astype(np.float32)  # [NP, FW]
        for b in range(BPC):
            g = assign[c][b]
            ix = idxs_all[g]
            cnt = len(ix)
            if cnt:
                flat = comp[pbase[b] : pbase[b] + cpbs[b], :].reshape(-1)
                out[g, ix] = flat[:cnt]
            else:
                out[g, :] = 1.0 / S  # softmax of all -1e10 is uniform
    if run_kwargs:
        kernel.last_results = res  # stash for the profiling harness
    return out
